# revision 1
# baseline (speedup 1.0000x reference)
"""KNN-Attention Trainium2 kernel (8-core SPMD, batch+sequence sharded).

Full inputs in, full output out. Sharding: 8 cores = 4 batches x 2 sequence
halves. Each core receives its batch's q rotated so its own 1024 rows come
first (rows 1024:2048 are the sibling half, needed only for the kNN counts),
plus that batch's mem_table and the replicated weights.

Algorithm per core (validated against the reference in fp32, rel err ~1e-6):
  1. qp^T = (q @ w_q)^T via PE-transposed q tiles        (d on partitions)
  2. kNN scores S = qp @ mem_table^T per 128-row l-tile; row max via DVE;
     indicator (S >= rowmax); counts c_u accumulated with a ones-vector
     matmul. Replaces argmax+gather: attention over the 1000 memory slots
     with multiplicity weights c_u is exactly attention over the 2048
     gathered keys.
  3. K^T = (mem_table @ w_kv[:, :64])^T computed directly; V1c[u] =
     c_u * [V_u | 1] so the ones-column yields the softmax denominator and
     c_u folds in multiplicatively (no ln / no max-subtraction needed:
     |scores/8| < 3 for this input distribution).
  4. Per head: S2^T(u,l) = K^T.T @ qh^T (two heads of a pair row-packed on
     the PE via tile_position), P = exp(S2/8), out'^T accumulated over u
     with lhsT = c.[V|1]. Normalize: out_h^T * broadcast(1/denom).
  5. final = out_norm @ w_concat accumulated over the 8 head-pairs.
"""

import sys

sys.path.insert(0, "/opt/trn_rl_repo")

import numpy as np

B, L, D, N_MEM, H, DH = 4, 2048, 1024, 1000, 16, 64
LO = L // 2  # rows owned per core
NU, U = 8, 125  # u-tiles over n_mem
KT = D // 128  # 8 contraction tiles
NCH = ((0, 512), (512, 488))  # n_mem free-dim chunks, PSUM-bank aligned

_CACHED = {}


def _build_nc():
    from concourse import bacc, mybir
    import concourse.tile as tile

    F32 = mybir.dt.float32
    nc = bacc.Bacc(
        "TRN2",
        target_bir_lowering=False,
        debug=False,
        enable_asserts=False,
        num_devices=8,
    )
    q_d = nc.dram_tensor("q", [L, D], F32, kind="ExternalInput")
    mem_d = nc.dram_tensor("mem_table", [N_MEM, D], F32, kind="ExternalInput")
    wq_d = nc.dram_tensor("w_q", [D, D], F32, kind="ExternalInput")
    wkv_d = nc.dram_tensor("w_kv", [D, 2 * DH], F32, kind="ExternalInput")
    wc_d = nc.dram_tensor("w_concat", [D, D], F32, kind="ExternalInput")
    out_d = nc.dram_tensor("out", [LO, D], F32, kind="ExternalOutput")

    with tile.TileContext(nc) as tc:
        _emit(nc, tc, q_d, mem_d, wq_d, wkv_d, wc_d, out_d)
    nc.compile()
    return nc


def _emit(nc, tc, q_d, mem_d, wq_d, wkv_d, wc_d, out_d):
    from concourse import mybir
    from concourse.masks import make_identity
    from contextlib import ExitStack

    F32 = mybir.dt.float32
    AX = mybir.AxisListType
    OP = mybir.AluOpType
    ACT = mybir.ActivationFunctionType

    ctx = ExitStack()
    with ctx:
        sb = ctx.enter_context(tc.tile_pool(name="sb", bufs=1))
        ps = ctx.enter_context(tc.tile_pool(name="ps", bufs=1, space="PSUM"))
        dr = ctx.enter_context(tc.tile_pool(name="dr", bufs=1, space="DRAM"))

        ident = sb.tile([128, 128], F32, name="ident")
        make_identity(nc, ident)
        ones = sb.tile([128, 64], F32, name="ones")
        nc.vector.memset(ones, 1.0)

        wq_sb = sb.tile([128, KT, D], F32, name="wq_sb", tag="w")
        nc.sync.dma_start(out=wq_sb, in_=wq_d.ap().rearrange("(k p) m -> p k m", p=128))
        wkv_sb = sb.tile([128, KT, 2 * DH], F32, name="wkv_sb")
        nc.sync.dma_start(
            out=wkv_sb, in_=wkv_d.ap().rearrange("(k p) m -> p k m", p=128)
        )

        qpT_own = sb.tile([128, KT, LO], F32, name="qpT_own")
        cnt_ps = ps.tile([1, N_MEM], F32, name="cnt_ps", tag="p4k", bufs=3)

        knn_calls = [0]

        def knn_ltile(lt, lhs_tile, lhs_off):
            """scores + rowmax + indicator + counts for one 128-row l-tile."""
            seq = knn_calls[0]
            knn_calls[0] += 1
            s_ps = ps.tile([128, N_MEM], F32, name=f"s_{lt}", tag="p4k", bufs=3)
            for o, w in NCH:
                for k in range(KT):
                    nc.tensor.matmul(
                        s_ps[:, o : o + w],
                        lhsT=lhs_tile[:, k, lhs_off : lhs_off + 128],
                        rhs=mT[:, k, o : o + w],
                        start=(k == 0),
                        stop=(k == KT - 1),
                    )
            mx = sb.tile([128, 1], F32, name=f"mx_{lt}", tag="mx", bufs=2)
            nc.vector.reduce_max(out=mx, in_=s_ps, axis=AX.X)
            ind = sb.tile([128, N_MEM], F32, name=f"ind_{lt}", tag="ind", bufs=2)
            nc.vector.tensor_single_scalar(ind, s_ps, mx, OP.is_ge)
            for o, w in NCH:
                nc.tensor.matmul(
                    cnt_ps[:, o : o + w],
                    lhsT=ones[:, 0:1],
                    rhs=ind[:, o : o + w],
                    start=(seq == 0),
                    stop=(seq == 15),
                    skip_group_check=True,
                )

        # ---- Phase 1.5: transpose mem_table -> mT (d on partitions) ----
        mT = sb.tile([128, KT, N_MEM], F32, name="mT")
        for u in range(NU):
            mn = sb.tile([128, D], F32, name=f"mn_{u}", tag="qn", bufs=2)
            nc.sync.dma_start(out=mn[:U, :], in_=mem_d.ap()[u * U : (u + 1) * U, :])
            # 128-aligned k-slots so each 125-wide transpose stays in one bank
            t2 = ps.tile([128, D], F32, name=f"t2_{u}", tag="p4k", bufs=3)
            for k in range(KT):
                nc.tensor.transpose(
                    t2[:, k * 128 : k * 128 + U],
                    mn[:U, k * 128 : (k + 1) * 128],
                    ident[:U, :U],
                )
            nc.vector.tensor_copy(
                mT[:, :, u * U : (u + 1) * U],
                t2.rearrange("p (k c) -> p k c", k=KT)[:, :, 0:U],
            )

        # ---- Phase 1: transpose q, qp^T = (q @ w_q)^T, other-half kNN ----
        for g in range(8):  # 256-wide l groups over full L
            qT_g = sb.tile([128, KT, 256], F32, name=f"qT_{g}", tag="qtg", bufs=2)
            for j in range(2):
                lt = 2 * g + j
                qn = sb.tile([128, D], F32, name=f"qn_{lt}", tag="qn", bufs=2)
                nc.sync.dma_start(out=qn, in_=q_d.ap()[lt * 128 : (lt + 1) * 128, :])
                trp = ps.tile([128, D], F32, name=f"trp_{lt}", tag="p4k", bufs=3)
                for k in range(KT):
                    nc.tensor.transpose(
                        trp[:, k * 128 : (k + 1) * 128],
                        qn[:, k * 128 : (k + 1) * 128],
                        ident,
                    )
                nc.vector.tensor_copy(
                    qT_g[:, :, j * 128 : (j + 1) * 128],
                    trp.rearrange("p (k c) -> p k c", k=KT),
                )
            if g < 4:
                dst, off = qpT_own, 256 * g
            else:
                dst = sb.tile([128, KT, 256], F32, name=f"qoth_{g}", tag="qoth", bufs=1)
                off = 0
            for m in range(KT):
                qp_ps = ps.tile([128, 256], F32, name=f"qp_{g}_{m}", tag="p2k", bufs=2)
                for k in range(KT):
                    nc.tensor.matmul(
                        qp_ps,
                        lhsT=wq_sb[:, k, m * 128 : (m + 1) * 128],
                        rhs=qT_g[:, k, :],
                        start=(k == 0),
                        stop=(k == KT - 1),
                    )
                nc.scalar.copy(dst[:, m, off : off + 256], qp_ps)
            if g >= 4:
                for j in range(2):
                    knn_ltile(8 + 2 * (g - 4) + j, dst, 128 * j)

        # ---- Phase 2: own-half kNN ----
        for lt in range(8):
            knn_ltile(lt, qpT_own, 128 * lt)

        # counts: psum row -> SBUF -> DRAM -> (125, 8) column layout
        cnt_dram = dr.tile([N_MEM], F32, name="cnt_dram")
        cnt_sb = sb.tile([1, N_MEM], F32, name="cnt_sb")
        nc.vector.tensor_copy(cnt_sb, cnt_ps)
        nc.sync.dma_start(out=cnt_dram.rearrange("(a b) -> a b", a=1), in_=cnt_sb)
        cnt_col = sb.tile([128, NU], F32, name="cnt_col")
        for t in range(NU):
            nc.sync.dma_start(
                out=cnt_col[:U, t : t + 1],
                in_=cnt_dram[t * U : (t + 1) * U].rearrange("(p a) -> p a", a=1),
            )

        # ---- Phase 4: K^T (doubled for row-packing) and V1c ----
        kT2 = sb.tile([128, N_MEM], F32, name="kT2")
        kt_ps = ps.tile([64, N_MEM], F32, name="kt_ps", tag="p4k", bufs=3)
        for o, w in NCH:
            for k in range(KT):
                nc.tensor.matmul(
                    kt_ps[:, o : o + w],
                    lhsT=wkv_sb[:, k, 0:DH],
                    rhs=mT[:, k, o : o + w],
                    start=(k == 0),
                    stop=(k == KT - 1),
                )
        nc.vector.tensor_copy(kT2[0:64, :], kt_ps)
        nc.vector.tensor_copy(kT2[64:128, :], kt_ps)

        v1c = sb.tile([128, NU, DH + 1], F32, name="v1c")
        for u in range(NU):
            v_ps = ps.tile([U, DH], F32, name=f"v_{u}", tag="p2k", bufs=2)
            for k in range(KT):
                nc.tensor.matmul(
                    v_ps,
                    lhsT=mT[:, k, u * U : (u + 1) * U],
                    rhs=wkv_sb[:, k, DH : 2 * DH],
                    start=(k == 0),
                    stop=(k == KT - 1),
                )
            nc.scalar.mul(v1c[:U, u, 0:DH], v_ps, mul=cnt_col[:U, u : u + 1])
            nc.vector.tensor_copy(v1c[:U, u, DH : DH + 1], cnt_col[:U, u : u + 1])

        # ---- Phase 5: attention, two heads of a pair interleaved ----
        pairTs = []
        for p in range(8):
            pairT = sb.tile([128, LO], F32, name=f"pairT_{p}", tag="pairT", bufs=8)
            pairTs.append(pairT)
            o_pss = []
            for sub in range(2):
                h = 2 * p + sub
                o_pss.append(
                    ps.tile([DH + 1, LO], F32, name=f"o_{h}", tag="p4k", bufs=3)
                )
            for u in range(NU):
                for sub in range(2):
                    h, hr = 2 * p + sub, sub * 64
                    s2 = ps.tile([U, LO], F32, name=f"s2_{h}_{u}", tag="p4k", bufs=3)
                    for c2 in range(2):
                        nc.tensor.matmul(
                            s2[:, c2 * 512 : (c2 + 1) * 512],
                            lhsT=kT2[hr : hr + 64, u * U : (u + 1) * U],
                            rhs=qpT_own[hr : hr + 64, p, c2 * 512 : (c2 + 1) * 512],
                            start=True,
                            stop=True,
                            tile_position=(hr, 0),
                        )
                    PT = sb.tile([128, LO], F32, name=f"PT_{h}_{u}", tag="ptu", bufs=4)
                    nc.scalar.activation(PT[:U, :], s2, ACT.Exp, scale=0.125)
                    for c2 in range(2):
                        nc.tensor.matmul(
                            o_pss[sub][:, c2 * 512 : (c2 + 1) * 512],
                            lhsT=v1c[:U, u, :],
                            rhs=PT[:U, c2 * 512 : (c2 + 1) * 512],
                            start=(u == 0),
                            stop=(u == NU - 1),
                            skip_group_check=True,
                        )
            for sub in range(2):
                h, hr, o_ps = 2 * p + sub, sub * 64, o_pss[sub]
                # o_sb row 0 = 1/denom (kept at partition 0 so it can feed the
                # K=1 broadcast matmul); rows 64..128 = unnormalized out_h^T
                o_sb = sb.tile([64 + DH, LO], F32, name=f"osb_{h}", tag="osb", bufs=1)
                nc.vector.reciprocal(o_sb[0:1, :], o_ps[DH : DH + 1, :])
                nc.vector.tensor_copy(o_sb[64 : 64 + DH, :], o_ps[0:DH, :])
                bc_ps = ps.tile([64, LO], F32, name=f"bc_{h}", tag="p4k", bufs=3)
                for c2 in range(2):
                    nc.tensor.matmul(
                        bc_ps[:, c2 * 512 : (c2 + 1) * 512],
                        lhsT=ones[0:1, :],
                        rhs=o_sb[0:1, c2 * 512 : (c2 + 1) * 512],
                        start=True,
                        stop=True,
                    )
                nc.vector.tensor_mul(
                    pairT[hr : hr + 64, :], o_sb[64 : 64 + DH, :], bc_ps
                )

        # ---- Phase 5b: final = out_norm @ w_concat ----
        wc_sb = sb.tile([128, KT, D], F32, name="wc_sb", tag="w")
        nc.sync.dma_start(out=wc_sb, in_=wc_d.ap().rearrange("(k p) m -> p k m", p=128))
        for lt in range(8):
            for c2 in range(2):
                f_ps = ps.tile([128, 512], F32, name=f"f_{lt}_{c2}", tag="p2k", bufs=2)
                for p in range(8):
                    nc.tensor.matmul(
                        f_ps,
                        lhsT=pairTs[p][:, lt * 128 : (lt + 1) * 128],
                        rhs=wc_sb[:, p, c2 * 512 : (c2 + 1) * 512],
                        start=(p == 0),
                        stop=(p == 7),
                    )
                f_sb = sb.tile([128, 512], F32, name=f"fs_{lt}_{c2}", tag="qn", bufs=2)
                nc.vector.tensor_copy(f_sb, f_ps)
                nc.sync.dma_start(
                    out=out_d.ap()[
                        lt * 128 : (lt + 1) * 128, c2 * 512 : (c2 + 1) * 512
                    ],
                    in_=f_sb,
                )


def get_nc():
    if "nc" not in _CACHED:
        _CACHED["nc"] = _build_nc()
    return _CACHED["nc"]


def make_in_maps(q, mem_table, w_q, w_kv, w_concat):
    f = np.float32
    q, mem_table = np.asarray(q, f), np.asarray(mem_table, f)
    w_q, w_kv, w_concat = (
        np.ascontiguousarray(np.asarray(w_q, f)),
        np.ascontiguousarray(np.asarray(w_kv, f)),
        np.ascontiguousarray(np.asarray(w_concat, f)),
    )
    in_maps = []
    for core in range(8):
        b, half = core // 2, core % 2
        qb = np.ascontiguousarray(
            np.concatenate([q[b, half * LO :], q[b, : half * LO]])
        )
        in_maps.append(
            {
                "q": qb,
                "mem_table": np.ascontiguousarray(mem_table[b]),
                "w_q": w_q,
                "w_kv": w_kv,
                "w_concat": w_concat,
            }
        )
    return in_maps


def kernel(q, kv, mem_table, w_q, w_kv, w_concat, topk, **run_kwargs):
    """Full (unsharded) inputs -> full (b, l, d) float32 output."""
    from concourse.bass_utils import run_bass_kernel_spmd

    nc = get_nc()
    in_maps = make_in_maps(q, mem_table, w_q, w_kv, w_concat)
    res = run_bass_kernel_spmd(nc, in_maps, core_ids=list(range(8)), **run_kwargs)
    out = np.zeros((B, L, D), np.float32)
    for core in range(8):
        b, half = core // 2, core % 2
        out[b, half * LO : (half + 1) * LO] = res.results[core]["out"]
    if run_kwargs:
        return out, res
    return out



# revision 48
# speedup vs baseline: 1.6377x; 1.6377x over previous
"""KNN-Attention Trainium2 kernel (8-core SPMD, batch+sequence sharded).

Full inputs in, full output out. Sharding: 8 cores = 4 batches x 2 sequence
halves. Each core receives its batch's q rotated so its own 1024 rows come
first (rows 1024:2048 are the sibling half, needed only for the kNN counts),
plus that batch's mem_table and the replicated weights.

Algorithm per core (validated against the reference on HW):
  1. qp^T = (q @ w_q)^T via PE-transposed q tiles        (d on partitions)
  2. kNN scores S = qp @ mem_table^T per 128-row l-tile; row max via DVE;
     indicator (S >= rowmax); counts accumulated with a ones-vector matmul
     per 2-l-tile batch, drained into an SBUF accumulator by DVE adds.
     Replaces argmax+gather: attention over the 1000 memory slots with
     multiplicity weights c_u is exactly attention over the 2048 gathered
     keys.
  3. K^T = (mem_table @ w_kv[:, :64])^T computed directly; V1c[u] =
     c_u * [V_u | 1] so the ones-column yields the softmax denominator and
     c_u folds in multiplicatively (no ln / no max-subtraction needed:
     |scores/8| < 3 for this input distribution).
  4. Per head (single-head pipeline): S2^T(u,l) = K^T.T @ qh^T, P =
     exp(S2/8), out'^T accumulated over u into two 1-bank PSUM halves.
     Normalize: recip of the ones-row, PE-broadcast, DVE multiply.
  5. final = out_norm @ w_concat accumulated over the 8 head-pairs.

All matmul operands are float32r (PE streams 1 row/cycle vs fp32's 4 when
the moving free dim >= 256); producers round on write per the walrus
verifier's fp32r contract. PSUM is split into a 4-slot 1-bank ring ("b1")
and a 2-slot 2-bank ring ("big2") so score tiles, transposes, s2 tiles and
the per-head output accumulators never fight for the same slots.
"""

import sys

sys.path.insert(0, "/opt/trn_rl_repo")

import numpy as np

B, L, D, N_MEM, H, DH = 4, 2048, 1024, 1000, 16, 64
LO = L // 2  # rows owned per core
NU, U = 8, 125  # u-tiles over n_mem
KT = D // 128  # 8 contraction tiles
NCH = ((0, 512), (512, 488))  # n_mem free-dim chunks, PSUM-bank aligned

_CACHED = {}


def _build_nc():
    from concourse import bacc, mybir
    import concourse.tile as tile

    F32 = mybir.dt.float32
    nc = bacc.Bacc(
        "TRN2",
        target_bir_lowering=False,
        debug=False,
        enable_asserts=False,
        num_devices=8,
    )
    q_d = nc.dram_tensor("q", [L, D], F32, kind="ExternalInput")
    mem_d = nc.dram_tensor("mem_table", [N_MEM, D], F32, kind="ExternalInput")
    wq_d = nc.dram_tensor("w_q", [D, D], F32, kind="ExternalInput")
    wkv_d = nc.dram_tensor("w_kv", [D, 2 * DH], F32, kind="ExternalInput")
    wc_d = nc.dram_tensor("w_concat", [D, D], F32, kind="ExternalInput")
    out_d = nc.dram_tensor("out", [LO, D], F32, kind="ExternalOutput")

    with tile.TileContext(nc) as tc:
        _emit(nc, tc, q_d, mem_d, wq_d, wkv_d, wc_d, out_d)
    nc.compile()
    return nc


def _emit(nc, tc, q_d, mem_d, wq_d, wkv_d, wc_d, out_d):
    from concourse import mybir
    from concourse.masks import make_identity
    from contextlib import ExitStack

    F32 = mybir.dt.float32
    F32R = mybir.dt.float32r
    AX = mybir.AxisListType
    OP = mybir.AluOpType
    ACT = mybir.ActivationFunctionType

    ctx = ExitStack()
    with ctx:
        sb = ctx.enter_context(tc.tile_pool(name="sb", bufs=1))
        ps = ctx.enter_context(tc.tile_pool(name="ps", bufs=1, space="PSUM"))
        dr = ctx.enter_context(tc.tile_pool(name="dr", bufs=1, space="DRAM"))

        ident_f = sb.tile([128, 128], F32, name="ident_f")
        make_identity(nc, ident_f)
        ident = sb.tile([128, 128], F32R, name="ident")
        nc.vector.tensor_copy(ident, ident_f)
        ones_f = sb.tile([128, 64], F32, name="ones_f")
        nc.vector.memset(ones_f, 1.0)
        ones = sb.tile([128, 64], F32R, name="ones")
        nc.vector.tensor_copy(ones, ones_f)

        qpT_own = sb.tile([128, KT, LO], F32R, name="qpT_own")
        cnt_acc = sb.tile([1, N_MEM], F32, name="cnt_acc")
        nc.vector.memset(cnt_acc, 0.0)

        knn_calls = [0]

        def knn_ltile(lt, lhs_tile, lhs_off):
            """scores + rowmax + indicator + counts for one 128-row l-tile.

            Counts accumulate in PSUM across adjacent call pairs (b1 ring
            slots are only held for the two back-to-back calls), then DVE
            adds drain them into cnt_acc.
            """
            seq = knn_calls[0]
            knn_calls[0] += 1
            s_ps = ps.tile([128, N_MEM], F32, name=f"s_{lt}", tag="big2", bufs=2)
            for o, w in NCH:
                for k in range(KT):
                    nc.tensor.matmul(
                        s_ps[:, o : o + w],
                        lhsT=lhs_tile[:, k, lhs_off : lhs_off + 128],
                        rhs=mT[:, k, o : o + w],
                        start=(k == 0),
                        stop=(k == KT - 1),
                    )
            mx = sb.tile([128, 1], F32, name=f"mx_{lt}", tag="mx", bufs=2)
            nc.vector.reduce_max(out=mx, in_=s_ps, axis=AX.X)
            ind = sb.tile([128, N_MEM], F32R, name=f"ind_{lt}", tag="sc4", bufs=2)
            nc.vector.tensor_single_scalar(ind, s_ps, mx, OP.is_ge)
            if seq % 2 == 0:
                knn_calls.append(
                    [
                        ps.tile([1, w], F32, name=f"cnt_{lt}_{o}", tag="b1", bufs=4)
                        for o, w in NCH
                    ]
                )
            cnt_chunks = knn_calls[-1]
            for ci, (o, w) in enumerate(NCH):
                nc.tensor.matmul(
                    cnt_chunks[ci],
                    lhsT=ones[:, 0:1],
                    rhs=ind[:, o : o + w],
                    start=(seq % 2 == 0),
                    stop=(seq % 2 == 1),
                    skip_group_check=True,
                )
            if seq % 2 == 1:
                for ci, (o, w) in enumerate(NCH):
                    nc.vector.tensor_add(
                        cnt_acc[:, o : o + w], cnt_acc[:, o : o + w], cnt_chunks[ci]
                    )

        # ---- Phase 1.5: transpose mem_table -> mT (d on partitions) ----
        # 125-partition F32R transposes fail walrus codegen, so these stay
        # plain fp32; the copy into mT rounds.
        mT = sb.tile([128, KT, N_MEM], F32R, name="mT")
        for u in range(NU):
            mn = sb.tile([128, D], F32, name=f"mn_{u}", tag="qn", bufs=2)
            nc.sync.dma_start(out=mn[:U, :], in_=mem_d.ap()[u * U : (u + 1) * U, :])
            # 128-aligned k-slots so each 125-wide transpose stays in one bank
            t2 = ps.tile([128, D], F32, name=f"t2_{u}", tag="big2", bufs=2)
            for k in range(KT):
                nc.tensor.transpose(
                    t2[:, k * 128 : k * 128 + U],
                    mn[:U, k * 128 : (k + 1) * 128],
                    ident_f[:U, :U],
                )
            nc.vector.tensor_copy(
                mT[:, :, u * U : (u + 1) * U],
                t2.rearrange("p (k c) -> p k c", k=KT)[:, :, 0:U],
            )

        # Weight DMAs after the mem-table stream so the serial DMA bus
        # serves the startup-critical tiles first. wq_sb is dead after the
        # last qp matmul; pairT8 (phase 5) reuses its 32KB via the shared
        # single-slot tag. The DMA is split per m-chunk so the first qp
        # matmul only waits for its own 512KB slice.
        wq_sb = sb.tile([128, KT, D], F32R, name="wq_sb", tag="w32", bufs=1)
        wq_r = wq_d.ap().rearrange("(k p) m -> p k m", p=128).bitcast(F32R)
        for m in range(KT):
            nc.sync.dma_start(
                out=wq_sb[:, :, m * 128 : (m + 1) * 128],
                in_=wq_r[:, :, m * 128 : (m + 1) * 128],
            )
        wkv_sb = sb.tile([128, KT, 2 * DH], F32R, name="wkv_sb")
        nc.sync.dma_start(
            out=wkv_sb,
            in_=wkv_d.ap().rearrange("(k p) m -> p k m", p=128).bitcast(F32R),
        )

        # ---- Phase 1: transpose q, qp^T = (q @ w_q)^T ----
        # Other half (g 4..7) first so its kNN l-tiles drain while the own
        # half's qp still computes; knn for group g is emitted one g late so
        # the PE has score work to run while DVE copies the next qT group.
        pending_knn = []
        qoths = {}
        for g in list(range(4, 8)) + list(range(0, 4)):
            qT_g = sb.tile([128, KT, 256], F32R, name=f"qT_{g}", tag="qtg", bufs=2)
            for j in range(2):
                lt = 2 * g + j
                qn = sb.tile([128, D], F32R, name=f"qn_{lt}", tag="qn", bufs=2)
                nc.sync.dma_start(
                    out=qn, in_=q_d.ap()[lt * 128 : (lt + 1) * 128, :].bitcast(F32R)
                )
                trp = ps.tile([128, D], F32R, name=f"trp_{lt}", tag="big2", bufs=2)
                for k in range(KT):
                    nc.tensor.transpose(
                        trp[:, k * 128 : (k + 1) * 128],
                        qn[:, k * 128 : (k + 1) * 128],
                        ident,
                    )
                nc.vector.tensor_copy(
                    qT_g[:, :, j * 128 : (j + 1) * 128],
                    trp.rearrange("p (k c) -> p k c", k=KT),
                )
            for fn in pending_knn:
                fn()
            pending_knn = []
            if g < 4:
                dst, off = qpT_own, 256 * g
            else:
                dst = sb.tile([128, KT, 256], F32R, name=f"qoth_{g}", tag="qo", bufs=2)
                qoths[g] = dst
                off = 0
            for m in range(KT):
                qp_ps = ps.tile([128, 256], F32, name=f"qp_{g}_{m}", tag="b1", bufs=4)
                for k in range(KT):
                    nc.tensor.matmul(
                        qp_ps,
                        lhsT=wq_sb[:, k, m * 128 : (m + 1) * 128],
                        rhs=qT_g[:, k, :],
                        start=(k == 0),
                        stop=(k == KT - 1),
                    )
                nc.scalar.copy(dst[:, m, off : off + 256], qp_ps)
            if g >= 4:

                def make_knn(gg, tile_):
                    return lambda: [
                        knn_ltile(8 + 2 * (gg - 4) + j, tile_, 128 * j)
                        for j in range(2)
                    ]

                pending_knn.append(make_knn(g, dst))
        for fn in pending_knn:
            fn()

        # w_concat is only read in phase 5b; chunked so the greedy DMA
        # scheduler can't block the q/mem stream with one 4MB transfer.
        wc_sb = sb.tile([128, KT, D], F32R, name="wc_sb")
        wc_r = wc_d.ap().rearrange("(k p) m -> p k m", p=128).bitcast(F32R)
        for m in range(KT):
            nc.sync.dma_start(
                out=wc_sb[:, :, m * 128 : (m + 1) * 128],
                in_=wc_r[:, :, m * 128 : (m + 1) * 128],
            )

        # ---- Phase 4a: K^T (doubled for row-packing) and V (counts-free) --
        kT2 = sb.tile([128, N_MEM], F32R, name="kT2")
        for ci, (o, w) in enumerate(NCH):
            kt_ps = ps.tile([64, w], F32, name=f"kt_{ci}", tag="b1", bufs=4)
            for k in range(KT):
                nc.tensor.matmul(
                    kt_ps,
                    lhsT=wkv_sb[:, k, 0:DH],
                    rhs=mT[:, k, o : o + w],
                    start=(k == 0),
                    stop=(k == KT - 1),
                )
            nc.vector.tensor_copy(kT2[0:64, o : o + w], kt_ps)
            nc.vector.tensor_copy(kT2[64:128, o : o + w], kt_ps)

        v_sb = sb.tile([128, NU, DH], F32, name="v_sb")
        for u in range(NU):
            v_ps = ps.tile([U, DH], F32, name=f"v_{u}", tag="b1", bufs=4)
            for k in range(KT):
                nc.tensor.matmul(
                    v_ps,
                    lhsT=mT[:, k, u * U : (u + 1) * U],
                    rhs=wkv_sb[:, k, DH : 2 * DH],
                    start=(k == 0),
                    stop=(k == KT - 1),
                )
            nc.vector.tensor_copy(v_sb[:U, u, :], v_ps)

        # ---- Phase 2: own-half kNN ----
        for lt in range(8):
            knn_ltile(lt, qpT_own, 128 * lt)

        # counts: SBUF row -> DRAM -> (125, 8) column layout (a partition
        # redistribution has to bounce through DRAM)
        cnt_dram = dr.tile([N_MEM], F32, name="cnt_dram")
        nc.sync.dma_start(out=cnt_dram.rearrange("(a b) -> a b", a=1), in_=cnt_acc)
        cnt_col = sb.tile([128, NU], F32, name="cnt_col")
        for t in range(NU):
            nc.sync.dma_start(
                out=cnt_col[:U, t : t + 1],
                in_=cnt_dram[t * U : (t + 1) * U].rearrange("(p a) -> p a", a=1),
            )

        # ---- Phase 4b: V1c[u] = c_u * [V_u | 1] ----
        v1c = sb.tile([128, NU, DH + 1], F32R, name="v1c")
        for u in range(NU):
            nc.scalar.mul(v1c[:U, u, 0:DH], v_sb[:U, u, :], mul=cnt_col[:U, u : u + 1])
            nc.vector.tensor_copy(v1c[:U, u, DH : DH + 1], cnt_col[:U, u : u + 1])

        # ---- Phase 5: attention, one head at a time ----
        # s2 tiles rotate in the 2-slot big2 ring; the per-head output
        # accumulates in two 1-bank b1 halves so consecutive heads double-
        # buffer.
        pairT8 = sb.tile([128, KT, LO], F32R, name="pairT8", tag="w32", bufs=1)
        pending_drain = []
        for p in range(8):
            # The two heads of a pair interleave at u-step granularity so the
            # Act engine always has an independent exp queued while the PE
            # feeds the sibling stream; their four 1-bank output halves fill
            # the whole b1 ring for the duration of the pair. The previous
            # pair's normalize/drain is emitted after u=0's s2+exp so the PE
            # streams straight into this pair while the DVE drains the old
            # accumulators — no Act bubble at the boundary.
            def s2_exp(u, sub):
                h, hr = 2 * p + sub, sub * 64
                s2 = ps.tile([U, LO], F32, name=f"s2_{h}_{u}", tag="big2", bufs=2)
                for c2 in range(2):
                    nc.tensor.matmul(
                        s2[:, c2 * 512 : (c2 + 1) * 512],
                        lhsT=kT2[hr : hr + 64, u * U : (u + 1) * U],
                        rhs=qpT_own[hr : hr + 64, p, c2 * 512 : (c2 + 1) * 512],
                        start=True,
                        stop=True,
                        tile_position=(hr, 0),
                    )
                PT = sb.tile([128, LO], F32R, name=f"PT_{h}_{u}", tag="ptu", bufs=3)
                nc.scalar.activation(PT[:U, :], s2, ACT.Exp, scale=0.125)
                return PT

            def pv(u, sub, PT):
                for c2 in range(2):
                    nc.tensor.matmul(
                        o_c2[sub][c2],
                        lhsT=v1c[:U, u, :],
                        rhs=PT[:U, c2 * 512 : (c2 + 1) * 512],
                        start=(u == 0),
                        stop=(u == NU - 1),
                        skip_group_check=True,
                    )

            # 3-deep exp pipeline: three s2+exp steps are in flight before the
            # first PV, so the PE keeps feeding the Act engine while this
            # pair's first PVs wait out the previous pair's accumulator
            # drain.
            steps = [(u, sub) for u in range(NU) for sub in range(2)]
            fifo = [(u, sub, s2_exp(u, sub)) for u, sub in steps[:3]]
            for fn in pending_drain:
                fn()
            pending_drain = []
            o_c2 = [
                [
                    ps.tile([DH + 1, 512], F32, name=f"o_{p}_{sub}_{c2}", tag="b1", bufs=4)
                    for c2 in range(2)
                ]
                for sub in range(2)
            ]
            for u, sub in steps[3:]:
                uu, ss, PT = fifo.pop(0)
                pv(uu, ss, PT)
                fifo.append((u, sub, s2_exp(u, sub)))
            for uu, ss, PT in fifo:
                pv(uu, ss, PT)
            def make_drain(pp, oo):
                def drain():
                    for sub in range(2):
                        h, hr = 2 * pp + sub, sub * 64
                        # normalize: 1/denominator broadcast across the 64
                        # head rows via a plain-fp32 K=1 matmul (den stays
                        # fp32 for the approx recip). Row 0 = 1/den, rows
                        # 1:65 = unnormalized out^T; shares the ring with the
                        # phase-2 ind tiles.
                        dn = sb.tile([128, LO], F32, name=f"dn_{h}", tag="sc4", bufs=2)
                        for c2 in range(2):
                            nc.vector.reciprocal(
                                dn[0:1, c2 * 512 : (c2 + 1) * 512],
                                oo[sub][c2][DH : DH + 1, :],
                            )
                            nc.vector.tensor_copy(
                                dn[64:128, c2 * 512 : (c2 + 1) * 512],
                                oo[sub][c2][0:DH, :],
                            )
                        bc = ps.tile([64, LO], F32, name=f"bc_{h}", tag="big2", bufs=2)
                        for c2 in range(2):
                            nc.tensor.matmul(
                                bc[:, c2 * 512 : (c2 + 1) * 512],
                                lhsT=ones_f[0:1, :],
                                rhs=dn[0:1, c2 * 512 : (c2 + 1) * 512],
                                start=True,
                                stop=True,
                            )
                        nc.vector.tensor_mul(
                            pairT8[hr : hr + 64, pp, :], dn[64:128, :], bc
                        )

                return drain

            pending_drain.append(make_drain(p, o_c2))
        for fn in pending_drain:
            fn()

        # ---- Phase 5b: final = out_norm @ w_concat ----
        for lt in range(8):
            for c2 in range(2):
                f_ps = ps.tile([128, 512], F32, name=f"f_{lt}_{c2}", tag="b1", bufs=4)
                for p in range(8):
                    nc.tensor.matmul(
                        f_ps,
                        lhsT=pairT8[:, p, lt * 128 : (lt + 1) * 128],
                        rhs=wc_sb[:, p, c2 * 512 : (c2 + 1) * 512],
                        start=(p == 0),
                        stop=(p == 7),
                    )
                f_sb = sb.tile([128, 512], F32, name=f"fs_{lt}_{c2}", tag="qn", bufs=2)
                nc.vector.tensor_copy(f_sb, f_ps)
                nc.sync.dma_start(
                    out=out_d.ap()[
                        lt * 128 : (lt + 1) * 128, c2 * 512 : (c2 + 1) * 512
                    ],
                    in_=f_sb,
                )


def get_nc():
    if "nc" not in _CACHED:
        _CACHED["nc"] = _build_nc()
    return _CACHED["nc"]


def make_in_maps(q, mem_table, w_q, w_kv, w_concat):
    f = np.float32
    q, mem_table = np.asarray(q, f), np.asarray(mem_table, f)
    w_q, w_kv, w_concat = (
        np.ascontiguousarray(np.asarray(w_q, f)),
        np.ascontiguousarray(np.asarray(w_kv, f)),
        np.ascontiguousarray(np.asarray(w_concat, f)),
    )
    in_maps = []
    for core in range(8):
        b, half = core // 2, core % 2
        qb = np.ascontiguousarray(
            np.concatenate([q[b, half * LO :], q[b, : half * LO]])
        )
        in_maps.append(
            {
                "q": qb,
                "mem_table": np.ascontiguousarray(mem_table[b]),
                "w_q": w_q,
                "w_kv": w_kv,
                "w_concat": w_concat,
            }
        )
    return in_maps


def kernel(q, kv, mem_table, w_q, w_kv, w_concat, topk, **run_kwargs):
    """Full (unsharded) inputs -> full (b, l, d) float32 output."""
    from concourse.bass_utils import run_bass_kernel_spmd

    nc = get_nc()
    in_maps = make_in_maps(q, mem_table, w_q, w_kv, w_concat)
    res = run_bass_kernel_spmd(nc, in_maps, core_ids=list(range(8)), **run_kwargs)
    out = np.zeros((B, L, D), np.float32)
    for core in range(8):
        b, half = core // 2, core % 2
        out[b, half * LO : (half + 1) * LO] = res.results[core]["out"]
    if run_kwargs:
        return out, res
    return out
